# revision 9
# baseline (speedup 1.0000x reference)
"""Trainium2 Bass kernel for the 4-layer sum/product circuit
(nn_KnowledgeLayer): h = enc(x); h = h[idx0].prod(1); h = h[idx1].sum(1);
h = h[idx2].prod(1); h = h[idx3].sum(1).

Strategy (shard the COMPOSED SLOT STREAM, not the batch):
  * Host composes the four index maps into TWO flat operand streams of
    32768 row-indices each into a 4098-row full-batch enc table
    ([x | 1-x | 0 | 1], built host-side as [4098, 1024] fp16).
  * Core c owns h3 rows [c*512, (c+1)*512) and gathers FULL 2KB rows
    at HBM line rate.  Chunks of [64, 128, 128, 128, 64] outputs keep
    the total DMA-instruction count (10 gathers + 1 meta load + 8
    output stores = 19) within the Tile DMA-semaphore pool, so no
    mid-program semaphore-recycle barrier serializes the tail.
  * Slot position g = j*128 + p with p = ii*2 + a and j = cb*2 + b
    (64-chunks) or j = cb*4 + b*2 + oo (128-chunks), so every tree
    reduction is a contiguous slice on DVE, and the final a-sum pairs
    adjacent PARTITIONS via a PE matmul with a [128, 64] pairing
    matrix (f32 PSUM); ACT drains PSUM to fp16; DMA writes fp16 rows
    (host upcasts to f32).

The bass program is identical for all 8 cores (pure SPMD); per-core
index streams differ via in_maps.
"""

import numpy as np

N_VARS = 2048
BATCH = 1024
NCORES = 8
TABLE = 2 * N_VARS + 2            # 4098
NOUT = 4096                       # h3 rows total
CORE_OUT = NOUT // NCORES         # 512 h3 rows per core
CHUNKS = (64, 128, 128, 128, 64)  # outputs per chunk
ICOLS = CORE_OUT * 8 // 16        # 256 idx columns per stream


# ----------------------------------------------------------------------------
# host-side index preparation
# ----------------------------------------------------------------------------

def _remap(e):
    """reference enc row -> our table row.
    table: [0,2048) = x[f], [2048,4096) = 1-x[f], 4096 = 0, 4097 = 1."""
    out = np.empty_like(e)
    out[e == 0] = 2 * N_VARS
    out[e == 1] = 2 * N_VARS + 1
    even = (e >= 2) & (e % 2 == 0)
    out[even] = (e[even] - 2) // 2
    odd = (e >= 3) & (e % 2 == 1)
    out[odd] = N_VARS + (e[odd] - 3) // 2
    return out


def _compose_indices(idx0, idx1, idx2, idx3):
    J = idx3.reshape(-1)              # [8192]  (i, a)   layer3 sum pairs
    K = idx2[J].reshape(-1)           # [16384] (i, a, b) layer2 prod pairs
    L = idx1[K].reshape(-1)           # [32768] (i, a, b, c) layer1 sum pairs
    AB = idx0[L]                      # [32768, 2]       layer0 prod pairs
    A = _remap(AB[:, 0].astype(np.int64))
    B = _remap(AB[:, 1].astype(np.int64))
    return A.reshape(NOUT, 2, 2, 2), B.reshape(NOUT, 2, 2, 2)


def _core_wrap(S, c):
    """Per-core chunked+wrapped int16 index tensor [128, ICOLS].

    Gather position within a chunk's call: g = j*128 + (ii*2 + a) with
    j = cb*2 + b for 64-output chunks and j = cb*4 + b*2 + oo for
    128-output chunks (output row = oo*64 + ii), so
    h1 = h0[:, :J/2]+h0[:, J/2:], h2 = h1[:, :J/4]*h1[:, J/4:J/2], and
    the final a-sum pairs adjacent partitions (PE matmul).
    SWDGE wraps each call's g-stream: idx[p16, s] = call[s*16 + p16].
    """
    Sc = S[c * CORE_OUT:(c + 1) * CORE_OUT]              # [512, 2, 2, 2]
    cols = []
    off = 0
    for cho in CHUNKS:
        Sk = Sc[off:off + cho]                           # [cho, a, b, cb]
        if cho == 64:
            Sk = Sk.transpose(3, 2, 0, 1)                # [cb, b, ii, a]
        else:
            Sk = Sk.reshape(2, 64, 2, 2, 2)              # [oo, ii, a, b, cb]
            Sk = Sk.transpose(4, 3, 0, 1, 2)             # [cb, b, oo, ii, a]
        stream = Sk.reshape(-1)                          # [cho*8]
        w = stream.reshape(-1, 16).T.astype(np.int16)    # [16, cho*8/16]
        cols.append(w)
        off += cho
    w = np.concatenate(cols, axis=1)                     # [16, ICOLS]
    return np.ascontiguousarray(np.tile(w, (8, 1)))      # [128, ICOLS]


# ----------------------------------------------------------------------------
# bass program (built once, cached)
# ----------------------------------------------------------------------------

_CACHED = {}


def _build_program():
    import concourse.bacc as bacc
    import concourse.mybir as mybir
    from concourse.tile import TileContext

    f32 = mybir.dt.float32
    f16 = mybir.dt.float16
    i16 = mybir.dt.int16

    nc = bacc.Bacc("TRN2", target_bir_lowering=False, debug=False,
                   num_swdge_queues=4)

    enc = nc.dram_tensor("enc", [TABLE, BATCH], f16, kind="ExternalInput")
    # meta packs idxa [128,256] i16, idxb [128,256] i16, pairs [128,64] f16
    meta = nc.dram_tensor("meta", [128, 2 * ICOLS + 64], i16,
                          kind="ExternalInput")
    out = nc.dram_tensor("out", [CORE_OUT, BATCH], f16, kind="ExternalOutput")

    with TileContext(nc) as tc:
        with tc.tile_pool(name="setup", bufs=1) as sp, \
             tc.tile_pool(name="gather", bufs=3) as gp, \
             tc.tile_pool(name="mid", bufs=2) as mp, \
             tc.tile_pool(name="hpsum", bufs=2, space="PSUM") as pp, \
             tc.tile_pool(name="outp", bufs=3) as outp:

            mt = sp.tile([128, 2 * ICOLS + 64], i16, tag="mt")
            nc.sync.dma_start(out=mt[:, :], in_=meta[:, :])
            ia = mt[:, 0:ICOLS]
            ib = mt[:, ICOLS:2 * ICOLS]
            pr = mt[:, 2 * ICOLS:].bitcast(f16)
            cnt1k = nc.gpsimd.to_reg(1024)
            cnt512 = nc.gpsimd.to_reg(512)

            icol = 0
            orow = 0
            for k, cho in enumerate(CHUNKS):
                nj = cho // 16           # 4 or 8 j-blocks
                slots = cho * 8          # per stream
                ic = slots // 16         # idx cols this chunk
                cnt = cnt512 if cho == 64 else cnt1k

                ga = gp.tile([128, 8, BATCH], f16, tag="ga")
                gb = gp.tile([128, 8, BATCH], f16, tag="gb")
                nc.gpsimd.dma_gather(
                    out_ap=ga[:, 0:nj, :], in_ap=enc[:, :],
                    idxs_ap=ia[:, icol:icol + ic],
                    num_idxs=slots, num_idxs_reg=cnt,
                    elem_size=BATCH, queue_num=(2 * k) % 4)
                nc.gpsimd.dma_gather(
                    out_ap=gb[:, 0:nj, :], in_ap=enc[:, :],
                    idxs_ap=ib[:, icol:icol + ic],
                    num_idxs=slots, num_idxs_reg=cnt,
                    elem_size=BATCH, queue_num=(2 * k + 1) % 4)
                icol += ic

                h0 = mp.tile([128, 8, BATCH], f16, tag="h0")
                h1 = mp.tile([128, 4, BATCH], f16, tag="h1")
                h2 = mp.tile([128, 2, BATCH], f16, tag="h2")
                ps = pp.tile([64, 2, BATCH], f32, tag="ps")
                ot = outp.tile([64, 2, BATCH], f16, tag="ot")
                noo = nj // 4            # 1 or 2 output groups
                nc.vector.tensor_mul(h0[:, 0:nj, :], ga[:, 0:nj, :],
                                     gb[:, 0:nj, :])
                nc.vector.tensor_add(h1[:, 0:nj // 2, :],
                                     h0[:, 0:nj // 2, :],
                                     h0[:, nj // 2:nj, :])
                nc.vector.tensor_mul(h2[:, 0:noo, :],
                                     h1[:, 0:noo, :],
                                     h1[:, noo:2 * noo, :])
                for oo in range(noo):
                    for half in range(2):
                        cs = slice(half * 512, (half + 1) * 512)
                        nc.tensor.matmul(
                            ps[:, oo, cs], lhsT=pr, rhs=h2[:, oo, cs],
                            start=True, stop=True)
                    nc.scalar.copy(ot[:, oo, :], ps[:, oo, :])
                    nc.sync.dma_start(
                        out=out[orow + oo * 64:orow + (oo + 1) * 64, :],
                        in_=ot[:, oo, :])
                orow += cho

    nc.compile()
    return nc


def _get_program():
    if "nc" not in _CACHED:
        _CACHED["nc"] = _build_program()
    return _CACHED["nc"]


# ----------------------------------------------------------------------------
# public entry point
# ----------------------------------------------------------------------------

def kernel(x, idx0, idx1, idx2, idx3, _trace=False, _trace_kwargs=None):
    from concourse.bass_utils import run_bass_kernel_spmd

    x = np.ascontiguousarray(np.asarray(x, dtype=np.float32))
    A, B = _compose_indices(
        np.asarray(idx0), np.asarray(idx1), np.asarray(idx2), np.asarray(idx3))

    enc = np.concatenate(
        [x, 1.0 - x,
         np.zeros((1, BATCH), np.float32),
         np.ones((1, BATCH), np.float32)], axis=0)
    enc = np.ascontiguousarray(enc.astype(np.float16))

    pairs = np.zeros((128, 64), np.float16)
    pairs[np.arange(128), np.arange(128) // 2] = 1.0

    nc = _get_program()
    in_maps = []
    for c in range(NCORES):
        mt = np.concatenate(
            [_core_wrap(A, c), _core_wrap(B, c), pairs.view(np.int16)], axis=1)
        in_maps.append({"enc": enc, "meta": np.ascontiguousarray(mt)})

    kwargs = {}
    if _trace:
        kwargs["trace"] = True
        if _trace_kwargs:
            kwargs.update(_trace_kwargs)
    res = run_bass_kernel_spmd(nc, in_maps, core_ids=list(range(NCORES)), **kwargs)
    outs = [res.results[c]["out"].astype(np.float32) for c in range(NCORES)]
    full = np.concatenate(outs, axis=0)
    if _trace:
        kernel.last_exec_time_ns = res.exec_time_ns
        kernel.last_profile = res.profile_json
    return full
